# revision 8
# baseline (speedup 1.0000x reference)
"""Trainium2 Bass kernel for nn_CustomModel_12953621365157 (gnn_message_passing).

Strategy
--------
Data-parallel over the batch axis: 8 cores, 512 batch columns each.

Per layer the reference does gather(edge_src) -> x0.5-weight-with-|g|==1-quirk
-> segment_sum(edge_dst) -> per-node activation.  Because the quirk depends
only on the gathered *value*, it folds into the source node:

    v_adj[n] = v[n] + c*(v[n]==1) - c*(v[n]==-1),   c = (1-w)/w
    summed   = w * (A_l @ v_adj)                    A_l[p,n] = #edges n->p

so each layer is a dense [P x N_l] x [N_l x B_s] matmul on the PE (float32r:
full-rate at N=512, ~1e-4 relative accuracy).  A_l is built host-side from the
edge lists.  Nodes of each layer are pre-sorted by activation id so the
per-node activation dispatch becomes a few partition-range ACT instructions
(cos = sin(x+pi/2), sigmoid = 0.5*tanh(x/2)+0.5, Gaussian = exp(-square(x)),
step on the DVE).  The sign trick: V tiles store -v_adj so the quirk chain is
2 fused scalar_tensor_tensor ops; all ACT scales are negated to compensate.
"""

import numpy as np

N_IN = 512
P = 512
L = 4
E = 32768
B = 4096
N_CORES = 8
BS = B // N_CORES  # 512 batch columns per core

# activation ids (order matches reference activations_dict)
LINEAR, STEP, SIN, COS, GAUSS, TANH, SIGMOID, ABS, INVERT, RELU = range(10)
# node sort order per layer: exp-set funcs first (gauss), then silu-set
# (sin/cos/tanh/sigmoid), then DVE/simple funcs.
FUNC_ORDER = [GAUSS, SIN, COS, TANH, SIGMOID, STEP, ABS, RELU, LINEAR, INVERT]
ORDER_RANK = {f: i for i, f in enumerate(FUNC_ORDER)}

TILE_BASE = [0, 4, 12, 24]  # first A-tile index of each layer; 40 tiles total
N_A_TILES = 40


def _preprocess(x, w, edge_src, edge_dst, act_ids):
    """Host-side: node sort per layer, dense A build, input quirk fold."""
    c = (1.0 - w) / w

    perms = []      # perms[l][p_sorted] = orig node j
    inv_perms = []  # inv_perms[l][orig j] = p_sorted
    segs = []       # segs[l][m] = list of (func_id, lo, hi) within chunk m
    for l in range(L):
        ids = np.asarray(act_ids[l])
        key = np.array([ORDER_RANK[int(i)] for i in ids])
        perm = np.argsort(key, kind="stable")
        inv = np.empty(P, np.int64)
        inv[perm] = np.arange(P)
        perms.append(perm)
        inv_perms.append(inv)
        ids_sorted = ids[perm]
        layer_segs = []
        for m in range(4):
            chunk = ids_sorted[m * 128:(m + 1) * 128]
            runs = []
            lo = 0
            for i in range(1, 129):
                if i == 128 or chunk[i] != chunk[lo]:
                    runs.append((int(chunk[lo]), lo, i))
                    lo = i
            layer_segs.append(runs)
        segs.append(layer_segs)

    inv_stack = np.stack(inv_perms)  # [L, P]

    a_pack = np.zeros((N_A_TILES * 128, P), np.float32)
    for l in range(L):
        src = np.asarray(edge_src[l]).astype(np.int64)
        dst = np.asarray(edge_dst[l]).astype(np.int64)
        g = src.copy()
        m = g >= N_IN
        lp = (g[m] - N_IN) // P
        j = (g[m] - N_IN) % P
        g[m] = N_IN + lp * P + inv_stack[lp, j]
        d = inv_perms[l][dst]
        np.add.at(a_pack, (TILE_BASE[l] * 128 + g, d), 1.0)

    xa = x.astype(np.float32)
    if c != 0.0:
        xa = xa + c * (xa == 1.0) - c * (xa == -1.0)
    xin = -xa  # V tiles hold -v_adj
    return a_pack, xin.astype(np.float32), perms, segs


DEBUG_DUMP = False


def _build_program(segs, w):
    import concourse.bass as bass
    import concourse.bacc as bacc
    import concourse.mybir as mybir
    import concourse.tile as tile

    dt = mybir.dt
    Act = mybir.ActivationFunctionType
    Alu = mybir.AluOpType
    W = float(w)

    # Cody-Waite split of 2*pi: c1/c2 short so k*c1, k*c2 are exact for
    # |k| < 2^12; c3 carries the remainder.
    def _trunc(x, bits):
        u = np.float32(x).view(np.uint32)
        mask = np.uint32(0xFFFFFFFF) << np.uint32(23 - bits)
        return float((u & mask).view(np.float32))

    TWO_PI = 2.0 * np.pi
    CW1 = _trunc(TWO_PI, 7)                    # 6.28125, exact
    CW2 = _trunc(TWO_PI - CW1, 12)
    CW3 = float(np.float32(TWO_PI - CW1 - CW2))
    INV_2PI = float(np.float32(1.0 / TWO_PI))
    PI_F = float(np.float32(np.pi))
    HALF_PI = float(np.float32(np.pi / 2))
    TWO_PI_F = float(np.float32(TWO_PI))
    c = (1.0 - W) / W
    fast_chain = (W == 0.5)

    nc = bacc.Bacc("TRN2", target_bir_lowering=False, debug=False,
                   num_devices=N_CORES)
    xin = nc.dram_tensor("xin", [N_IN, BS], dt.float32r,
                         kind="ExternalInput").ap()
    a_d = nc.dram_tensor("amat", [N_A_TILES * 128, P], dt.float32r,
                         kind="ExternalInput").ap()
    out_d = nc.dram_tensor("out", [P, BS], dt.float32,
                           kind="ExternalOutput").ap()
    dbg = {}
    if DEBUG_DUMP:
        for nm in ["k", "z", "r", "wrap"]:
            dbg[nm] = nc.dram_tensor(f"dbg_{nm}", [128, BS], dt.float32,
                                     kind="ExternalOutput").ap()
        for l in range(L - 1):
            dbg[("vraw", l)] = nc.dram_tensor(
                f"dbg_vraw{l}", [P, BS], dt.float32, kind="ExternalOutput").ap()
            dbg[("vadj", l)] = nc.dram_tensor(
                f"dbg_vadj{l}", [P, BS], dt.float32, kind="ExternalOutput").ap()

    with tile.TileContext(nc) as tc:
        with tc.tile_pool(name="Ap", bufs=1) as apool, \
             tc.tile_pool(name="Vp", bufs=1) as vpool, \
             tc.tile_pool(name="raw", bufs=5) as rpool, \
             tc.tile_pool(name="ps", bufs=8, space="PSUM") as ppool:

            # input node values (already quirk-folded & negated on host)
            V = []
            for t in range(4):
                vt = vpool.tile([128, BS], dt.float32r, name=f"v{t}")
                nc.sync.dma_start(vt[:], xin[t * 128:(t + 1) * 128, :])
                V.append(vt)

            A = {}
            for l in range(L):
                for k in range(4 + 4 * l):
                    at = apool.tile([128, P], dt.float32r, name=f"a{l}_{k}")
                    r0 = (TILE_BASE[l] + k) * 128
                    nc.sync.dma_start(at[:], a_d[r0:r0 + 128, :])
                    A[(l, k)] = at

            for l in range(L):
                nk = 4 + 4 * l
                psums = []
                for m in range(4):
                    ps = ppool.tile([128, BS], dt.float32, name="ps")
                    for k in range(nk):
                        nc.tensor.matmul(
                            ps[:], A[(l, k)][:, m * 128:(m + 1) * 128],
                            V[k][:], start=(k == 0), stop=(k == nk - 1))
                    psums.append(ps)

                # Engine instructions must start at partition 0/32/64/96
                # and not cross their aligned block end.  Each segment is
                # extended down to a 32-aligned start and split into valid
                # "buddy" pieces; segments are emitted in DESCENDING partition
                # order so the true owner of every overlap region writes last.
                # Gaussian (the only exp-table-set user) is emitted after all
                # silu-set segments of the layer: 2 ACT table loads per layer.
                def _pieces(lo, hi):
                    p = (lo // 32) * 32
                    out = []
                    while p < hi:
                        end = min(hi, 64) if p == 32 else hi
                        out.append((p, end))
                        p = end
                    return out

                vraws, tmps = [], []
                for m in range(4):
                    vraw = rpool.tile([128, BS], dt.float32, name="vraw")
                    tmp = rpool.tile([128, BS], dt.float32, name="tmp")
                    vraws.append(vraw)
                    tmps.append(tmp)
                # sin/cos need |arg| <= pi (the Sin spline only covers
                # |x| < 4): per chunk containing sin/cos nodes, compute the
                # Cody-Waite-reduced argument r = z - 2*pi*round-ish(z/2pi)
                # on the full tile (DVE cost is partition-count independent),
                # then each sin/cos segment wraps (+pi/2 for cos) into
                # [-pi, pi] and applies Sin with no bias.
                rtiles = {}
                for m in range(4):
                    if not any(f in (SIN, COS) for f, _, _ in segs[l][m]):
                        continue
                    ps = psums[m]
                    sq = rpool.tile([128, BS], dt.float32, name="sq", bufs=2)
                    si = rpool.tile([128, BS], dt.int32, name="si", bufs=2)
                    sz = rpool.tile([128, BS], dt.float32, name="sz", bufs=2)
                    sr = rpool.tile([128, BS], dt.float32, name="sr", bufs=2)
                    nc.vector.tensor_scalar(sq[:], ps[:], -W * INV_2PI, None,
                                            Alu.mult)
                    nc.vector.tensor_copy(si[:], sq[:])   # f32 -> i32
                    nc.vector.tensor_copy(sq[:], si[:])   # i32 -> f32 (= k)
                    nc.vector.tensor_scalar(sz[:], ps[:], -W, None, Alu.mult)
                    nc.vector.cody_waite_cascade(sr[:], sz[:], sq[:],
                                                 CW1, CW2, CW3)
                    # custom DVE ops silently no-op on partition-offset APs:
                    # do both wraps full-tile, slice only in the ACT reads.
                    nc.vector.add_range_wrap(sz[:], sr[:], 0.0, PI_F,
                                             TWO_PI_F)
                    if any(f == COS for f, _, _ in segs[l][m]):
                        nc.vector.add_range_wrap(sq[:], sr[:], HALF_PI, PI_F,
                                                 TWO_PI_F)
                    rtiles[m] = (sz, sq)

                for m in range(4):
                    ps, vraw, tmp = psums[m], vraws[m], tmps[m]
                    for fid, slo, shi in reversed(segs[l][m]):
                      if fid == GAUSS:
                        continue
                      for lo, hi in _pieces(slo, shi):
                        s = np.s_[lo:hi, :]
                        if fid in (SIN, COS):
                            wsin, wcos = rtiles[m]
                            src_t = wsin if fid == SIN else wcos
                            nc.scalar.activation(vraw[s], src_t[s], Act.Sin,
                                                 scale=1.0)
                        elif fid == TANH:
                            nc.scalar.activation(vraw[s], ps[s], Act.Tanh,
                                                 scale=-W)
                        elif fid == SIGMOID:
                            nc.scalar.activation(tmp[s], ps[s], Act.Tanh,
                                                 scale=-W / 2)
                            nc.vector.tensor_scalar(vraw[s], tmp[s], 0.5, 0.5,
                                                    Alu.mult, Alu.add)
                        elif fid == STEP:
                            # step(S_true) = +1 iff S_psum <= 0
                            nc.vector.tensor_scalar(tmp[s], ps[s], 0.0, None,
                                                    Alu.is_le)
                            nc.vector.tensor_scalar(vraw[s], tmp[s], 2.0, 1.0,
                                                    Alu.mult, Alu.subtract)
                        elif fid == ABS:
                            nc.scalar.activation(vraw[s], ps[s], Act.Abs,
                                                 scale=-W)
                        elif fid == INVERT:
                            nc.scalar.activation(vraw[s], ps[s], Act.Copy,
                                                 scale=W)
                        elif fid == LINEAR:
                            nc.scalar.activation(vraw[s], ps[s], Act.Copy,
                                                 scale=-W)
                        elif fid == RELU:
                            nc.scalar.activation(vraw[s], ps[s], Act.Relu,
                                                 scale=-W)
                        else:
                            raise ValueError(fid)

                # Gaussian pass (exp table set), rewrites any clobbered overlap
                for m in range(4):
                    ps, vraw, tmp = psums[m], vraws[m], tmps[m]
                    for fid, slo, shi in segs[l][m]:
                        if fid != GAUSS:
                            continue
                        for lo, hi in _pieces(slo, shi):
                            s = np.s_[lo:hi, :]
                            nc.scalar.activation(tmp[s], ps[s], Act.Square,
                                                 scale=-W)
                            nc.scalar.activation(vraw[s], tmp[s], Act.Exp,
                                                 scale=-1.0)

                for m in range(4):
                    ps, vraw, tmp = psums[m], vraws[m], tmps[m]
                    if l < L - 1:
                        vt = vpool.tile([128, BS], dt.float32r,
                                        name=f"v{4 + 4 * l + m}")
                        if fast_chain:
                            nc.vector.scalar_tensor_tensor(
                                tmp[:], vraw[:], 1.0, vraw[:],
                                Alu.is_equal, Alu.add)
                            nc.vector.scalar_tensor_tensor(
                                vt[:], vraw[:], -1.0, tmp[:],
                                Alu.is_equal, Alu.subtract)
                        else:
                            m1c = rpool.tile([128, BS], dt.float32, name="m1c")
                            nc.vector.tensor_scalar(m1c[:], vraw[:], 1.0, c,
                                                    Alu.is_equal, Alu.mult)
                            nc.vector.tensor_tensor(tmp[:], m1c[:], vraw[:],
                                                    Alu.add)
                            nc.vector.tensor_scalar(m1c[:], vraw[:], -1.0, c,
                                                    Alu.is_equal, Alu.mult)
                            nc.vector.tensor_tensor(vt[:], m1c[:], tmp[:],
                                                    Alu.subtract)
                        V.append(vt)
                        if DEBUG_DUMP:
                            nc.sync.dma_start(
                                dbg[("vraw", l)][m * 128:(m + 1) * 128, :],
                                vraw[:])
                            nc.sync.dma_start(
                                dbg[("vadj", l)][m * 128:(m + 1) * 128, :],
                                vt[:].bitcast(dt.float32))
                    else:
                        nc.sync.dma_start(out_d[m * 128:(m + 1) * 128, :],
                                          vraw[:])
    nc.compile()
    return nc


_CACHE = {}


def _get_program(segs_key, segs, w):
    key = (segs_key, float(w))
    if key not in _CACHE:
        _CACHE[key] = _build_program(segs, w)
    return _CACHE[key]


def kernel(x, shared_weight, edge_src, edge_dst, act_ids):
    from concourse.bass_utils import run_bass_kernel_spmd

    w = float(np.asarray(shared_weight))
    assert w != 0.0
    a_pack, xin, perms, segs = _preprocess(
        np.asarray(x), w, np.asarray(edge_src), np.asarray(edge_dst),
        np.asarray(act_ids))

    segs_key = tuple(tuple(tuple(r) for r in lm) for lseg in segs for lm in lseg)
    nc = _get_program(segs_key, segs, w)

    in_maps = [
        {"xin": np.ascontiguousarray(xin[:, cid * BS:(cid + 1) * BS]),
         "amat": a_pack}
        for cid in range(N_CORES)
    ]
    res = run_bass_kernel_spmd(nc, in_maps, core_ids=list(range(N_CORES)))
    out_sorted = np.concatenate([res.results[cid]["out"]
                                 for cid in range(N_CORES)], axis=1)
    out = np.empty_like(out_sorted)
    out[perms[L - 1]] = out_sorted
    return out.astype(np.float32)


# revision 13
# speedup vs baseline: 1.2617x; 1.2617x over previous
"""Trainium2 Bass kernel for nn_CustomModel_12953621365157 (gnn_message_passing).

Strategy
--------
Data-parallel over the batch axis: 8 cores, 512 batch columns each.

Per layer the reference does gather(edge_src) -> x0.5-weight-with-|g|==1-quirk
-> segment_sum(edge_dst) -> per-node activation.  Because the quirk depends
only on the gathered *value*, it folds into the source node:

    v_adj[n] = v[n] + c*(v[n]==1) - c*(v[n]==-1),   c = (1-w)/w
    summed   = w * (A_l @ v_adj)                    A_l[p,n] = #edges n->p

so each layer is a dense [P x N_l] x [N_l x B_s] matmul on the PE (float32r:
full-rate at N=512, ~1e-4 relative accuracy).  A_l is built host-side from the
edge lists.  Nodes of each layer are pre-sorted by activation id so the
per-node activation dispatch becomes a few partition-range ACT instructions
(cos = sin(x+pi/2), sigmoid = 0.5*tanh(x/2)+0.5, Gaussian = exp(-square(x)),
step on the DVE).  The sign trick: V tiles store -v_adj so the quirk chain is
2 fused scalar_tensor_tensor ops; all ACT scales are negated to compensate.
"""

import numpy as np

N_IN = 512
P = 512
L = 4
E = 32768
B = 4096
N_CORES = 8
BS = B // N_CORES  # 512 batch columns per core

# activation ids (order matches reference activations_dict)
LINEAR, STEP, SIN, COS, GAUSS, TANH, SIGMOID, ABS, INVERT, RELU = range(10)
# node sort order per layer: exp-set funcs first (gauss), then silu-set
# (sin/cos/tanh/sigmoid), then DVE/simple funcs.
FUNC_ORDER = [GAUSS, TANH, SIGMOID, STEP, ABS, RELU, LINEAR, INVERT, SIN, COS]
ORDER_RANK = {f: i for i, f in enumerate(FUNC_ORDER)}

TILE_BASE = [0, 4, 12, 24]  # first A-tile index of each layer; 40 tiles total
N_A_TILES = 40


def _preprocess(x, w, edge_src, edge_dst, act_ids):
    """Host-side: node sort per layer, dense A build, input quirk fold."""
    c = (1.0 - w) / w

    perms = []      # perms[l][p_sorted] = orig node j
    inv_perms = []  # inv_perms[l][orig j] = p_sorted
    segs = []       # segs[l][m] = list of (func_id, lo, hi) within chunk m
    for l in range(L):
        ids = np.asarray(act_ids[l])
        key = np.array([ORDER_RANK[int(i)] for i in ids])
        perm = np.argsort(key, kind="stable")
        inv = np.empty(P, np.int64)
        inv[perm] = np.arange(P)
        perms.append(perm)
        inv_perms.append(inv)
        ids_sorted = ids[perm]
        layer_segs = []
        for m in range(4):
            chunk = ids_sorted[m * 128:(m + 1) * 128]
            runs = []
            lo = 0
            for i in range(1, 129):
                if i == 128 or chunk[i] != chunk[lo]:
                    runs.append((int(chunk[lo]), lo, i))
                    lo = i
            layer_segs.append(runs)
        segs.append(layer_segs)

    inv_stack = np.stack(inv_perms)  # [L, P]

    a_pack = np.zeros((N_A_TILES * 128, P), np.float32)
    for l in range(L):
        src = np.asarray(edge_src[l]).astype(np.int64)
        dst = np.asarray(edge_dst[l]).astype(np.int64)
        g = src.copy()
        m = g >= N_IN
        lp = (g[m] - N_IN) // P
        j = (g[m] - N_IN) % P
        g[m] = N_IN + lp * P + inv_stack[lp, j]
        d = inv_perms[l][dst]
        np.add.at(a_pack, (TILE_BASE[l] * 128 + g, d), 1.0)

    xa = x.astype(np.float32)
    if c != 0.0:
        xa = xa + c * (xa == 1.0) - c * (xa == -1.0)
    xin = -xa  # V tiles hold -v_adj
    return a_pack, xin.astype(np.float32), perms, segs


DEBUG_DUMP = False
ACT_CHAIN = True


def _build_program(segs, w):
    import concourse.bass as bass
    import concourse.bacc as bacc
    import concourse.mybir as mybir
    import concourse.tile as tile
    from concourse.tile_rust import add_dep_helper

    dt = mybir.dt
    Act = mybir.ActivationFunctionType
    Alu = mybir.AluOpType
    W = float(w)

    # Cody-Waite split of 2*pi: c1/c2 short so k*c1, k*c2 are exact for
    # |k| < 2^12; c3 carries the remainder.
    def _trunc(x, bits):
        u = np.float32(x).view(np.uint32)
        mask = np.uint32(0xFFFFFFFF) << np.uint32(23 - bits)
        return float((u & mask).view(np.float32))

    TWO_PI = 2.0 * np.pi
    CW1 = _trunc(TWO_PI, 7)                    # 6.28125, exact
    CW2 = _trunc(TWO_PI - CW1, 12)
    CW3 = float(np.float32(TWO_PI - CW1 - CW2))
    INV_2PI = float(np.float32(1.0 / TWO_PI))
    PI_F = float(np.float32(np.pi))
    HALF_PI = float(np.float32(np.pi / 2))
    TWO_PI_F = float(np.float32(TWO_PI))
    c = (1.0 - W) / W
    fast_chain = (W == 0.5)

    nc = bacc.Bacc("TRN2", target_bir_lowering=False, debug=False,
                   num_devices=N_CORES)
    xin = nc.dram_tensor("xin", [N_IN, BS], dt.float32r,
                         kind="ExternalInput").ap()
    a_d = nc.dram_tensor("amat", [N_A_TILES * 128, P], dt.float32r,
                         kind="ExternalInput").ap()
    out_d = nc.dram_tensor("out", [P, BS], dt.float32,
                           kind="ExternalOutput").ap()
    dbg = {}
    if DEBUG_DUMP:
        for nm in ["k", "z", "r", "wrap"]:
            dbg[nm] = nc.dram_tensor(f"dbg_{nm}", [128, BS], dt.float32,
                                     kind="ExternalOutput").ap()
        for l in range(L - 1):
            dbg[("vraw", l)] = nc.dram_tensor(
                f"dbg_vraw{l}", [P, BS], dt.float32, kind="ExternalOutput").ap()
            dbg[("vadj", l)] = nc.dram_tensor(
                f"dbg_vadj{l}", [P, BS], dt.float32, kind="ExternalOutput").ap()

    with tile.TileContext(nc) as tc:
        with tc.tile_pool(name="Ap", bufs=1) as apool, \
             tc.tile_pool(name="Vp", bufs=1) as vpool, \
             tc.tile_pool(name="raw", bufs=5) as rpool, \
             tc.tile_pool(name="ps", bufs=8, space="PSUM") as ppool:

            # input node values (already quirk-folded & negated on host)
            V = []
            for t in range(4):
                vt = vpool.tile([128, BS], dt.float32r, name=f"v{t}")
                nc.sync.dma_start(vt[:], xin[t * 128:(t + 1) * 128, :])
                V.append(vt)

            A = {}
            for l in range(L):
                for k in range(4 + 4 * l):
                    at = apool.tile([128, P], dt.float32r, name=f"a{l}_{k}")
                    r0 = (TILE_BASE[l] + k) * 128
                    nc.sync.dma_start(at[:], a_d[r0:r0 + 128, :])
                    A[(l, k)] = at

            for l in range(L):
                nk = 4 + 4 * l
                psums = []
                for m in range(4):
                    ps = ppool.tile([128, BS], dt.float32, name="ps")
                    for k in range(nk):
                        nc.tensor.matmul(
                            ps[:], A[(l, k)][:, m * 128:(m + 1) * 128],
                            V[k][:], start=(k == 0), stop=(k == nk - 1))
                    psums.append(ps)

                # Engine instructions must start at partition 0/32/64/96
                # and not cross their aligned block end.  Each segment is
                # extended down to a 32-aligned start and split into valid
                # "buddy" pieces; segments are emitted in DESCENDING partition
                # order so the true owner of every overlap region writes last.
                # Gaussian (the only exp-table-set user) is emitted after all
                # silu-set segments of the layer: 2 ACT table loads per layer.
                def _pieces(lo, hi):
                    p = (lo // 32) * 32
                    out = []
                    while p < hi:
                        end = min(hi, 64) if p == 32 else hi
                        out.append((p, end))
                        p = end
                    return out

                vraws, tmps = [], []
                for m in range(4):
                    vraw = rpool.tile([128, BS], dt.float32, name="vraw")
                    tmp = rpool.tile([128, BS], dt.float32, name="tmp")
                    vraws.append(vraw)
                    tmps.append(tmp)
                # sin/cos need |arg| <= pi (the Sin spline only covers
                # |x| < 4): per chunk containing sin/cos nodes, compute the
                # Cody-Waite-reduced argument r = z - 2*pi*round-ish(z/2pi)
                # on the full tile (DVE cost is partition-count independent),
                # then each sin/cos segment wraps (+pi/2 for cos) into
                # [-pi, pi] and applies Sin with no bias.
                rtiles = {}
                for m in range(4):
                    if not any(f in (SIN, COS) for f, _, _ in segs[l][m]):
                        continue
                    ps = psums[m]
                    sq = rpool.tile([128, BS], dt.float32, name="sq", bufs=2)
                    si = rpool.tile([128, BS], dt.int32, name="si", bufs=2)
                    sz = rpool.tile([128, BS], dt.float32, name="sz", bufs=2)
                    sr = rpool.tile([128, BS], dt.float32, name="sr", bufs=2)
                    nc.vector.tensor_scalar(sq[:], ps[:], -W * INV_2PI, None,
                                            Alu.mult)
                    nc.vector.tensor_copy(si[:], sq[:])   # f32 -> i32
                    nc.vector.tensor_copy(sq[:], si[:])   # i32 -> f32 (= k)
                    nc.vector.tensor_scalar(sz[:], ps[:], -W, None, Alu.mult)
                    nc.vector.cody_waite_cascade(sr[:], sz[:], sq[:],
                                                 CW1, CW2, CW3)
                    # custom DVE ops silently no-op on partition-offset APs:
                    # do both wraps full-tile, slice only in the ACT reads.
                    nc.vector.add_range_wrap(sz[:], sr[:], 0.0, PI_F,
                                             TWO_PI_F)
                    if any(f == COS for f, _, _ in segs[l][m]):
                        nc.vector.add_range_wrap(sq[:], sr[:], HALF_PI, PI_F,
                                                 TWO_PI_F)
                    rtiles[m] = (sz, sq)

                # Single descending pass per chunk (chunks ascending):
                # with sin/cos sorted to the TOP partitions and gauss at the
                # bottom, emission order doubles as the table-set grouping
                # ([exp-set funcs + gauss] low chunks, [trig sin/cos] last
                # chunk) -> 2 ACT table loads per layer.  The add_dep chain
                # pins the ACT stream to this order.
                act_chain = []
                for m in range(4):
                    ps, vraw, tmp = psums[m], vraws[m], tmps[m]
                    for fid, slo, shi in reversed(segs[l][m]):
                      for lo, hi in _pieces(slo, shi):
                        s = np.s_[lo:hi, :]
                        if fid == GAUSS:
                            act_chain.append(nc.scalar.activation(
                                tmp[s], ps[s], Act.Square, scale=-W))
                            act_chain.append(nc.scalar.activation(
                                vraw[s], tmp[s], Act.Exp, scale=-1.0))
                        elif fid in (SIN, COS):
                            wsin, wcos = rtiles[m]
                            src_t = wsin if fid == SIN else wcos
                            act_chain.append(nc.scalar.activation(
                                vraw[s], src_t[s], Act.Sin, scale=1.0))
                        elif fid == TANH:
                            act_chain.append(nc.scalar.activation(
                                vraw[s], ps[s], Act.Tanh, scale=-W))
                        elif fid == SIGMOID:
                            act_chain.append(nc.scalar.activation(
                                tmp[s], ps[s], Act.Tanh, scale=-W / 2))
                            nc.vector.tensor_scalar(vraw[s], tmp[s], 0.5, 0.5,
                                                    Alu.mult, Alu.add)
                        elif fid == STEP:
                            # step(S_true) = +1 iff S_psum <= 0
                            nc.vector.tensor_scalar(tmp[s], ps[s], 0.0, None,
                                                    Alu.is_le)
                            nc.vector.tensor_scalar(vraw[s], tmp[s], 2.0, 1.0,
                                                    Alu.mult, Alu.subtract)
                        elif fid == ABS:
                            act_chain.append(nc.scalar.activation(
                                vraw[s], ps[s], Act.Abs, scale=-W))
                        elif fid == INVERT:
                            nc.vector.tensor_scalar(vraw[s], ps[s], W, None,
                                                    Alu.mult)
                        elif fid == LINEAR:
                            nc.vector.tensor_scalar(vraw[s], ps[s], -W, None,
                                                    Alu.mult)
                        elif fid == RELU:
                            act_chain.append(nc.scalar.activation(
                                vraw[s], ps[s], Act.Relu, scale=-W))
                        else:
                            raise ValueError(fid)

                if ACT_CHAIN:
                    # add_dep_helper(x, y) == "x waits on y"
                    for a, b in zip(act_chain, act_chain[1:]):
                        add_dep_helper(b.ins, a.ins, sync=False,
                                       reason="act table order")

                for m in range(4):
                    ps, vraw, tmp = psums[m], vraws[m], tmps[m]
                    if l < L - 1:
                        vt = vpool.tile([128, BS], dt.float32r,
                                        name=f"v{4 + 4 * l + m}")
                        if fast_chain:
                            nc.vector.scalar_tensor_tensor(
                                tmp[:], vraw[:], 1.0, vraw[:],
                                Alu.is_equal, Alu.add)
                            nc.vector.scalar_tensor_tensor(
                                vt[:], vraw[:], -1.0, tmp[:],
                                Alu.is_equal, Alu.subtract)
                        else:
                            m1c = rpool.tile([128, BS], dt.float32, name="m1c")
                            nc.vector.tensor_scalar(m1c[:], vraw[:], 1.0, c,
                                                    Alu.is_equal, Alu.mult)
                            nc.vector.tensor_tensor(tmp[:], m1c[:], vraw[:],
                                                    Alu.add)
                            nc.vector.tensor_scalar(m1c[:], vraw[:], -1.0, c,
                                                    Alu.is_equal, Alu.mult)
                            nc.vector.tensor_tensor(vt[:], m1c[:], tmp[:],
                                                    Alu.subtract)
                        V.append(vt)
                        if DEBUG_DUMP:
                            nc.sync.dma_start(
                                dbg[("vraw", l)][m * 128:(m + 1) * 128, :],
                                vraw[:])
                            nc.sync.dma_start(
                                dbg[("vadj", l)][m * 128:(m + 1) * 128, :],
                                vt[:].bitcast(dt.float32))
                    else:
                        nc.sync.dma_start(out_d[m * 128:(m + 1) * 128, :],
                                          vraw[:])
    nc.compile()
    return nc


_CACHE = {}


def _get_program(segs_key, segs, w):
    key = (segs_key, float(w))
    if key not in _CACHE:
        _CACHE[key] = _build_program(segs, w)
    return _CACHE[key]


def kernel(x, shared_weight, edge_src, edge_dst, act_ids):
    from concourse.bass_utils import run_bass_kernel_spmd

    w = float(np.asarray(shared_weight))
    assert w != 0.0
    a_pack, xin, perms, segs = _preprocess(
        np.asarray(x), w, np.asarray(edge_src), np.asarray(edge_dst),
        np.asarray(act_ids))

    segs_key = tuple(tuple(tuple(r) for r in lm) for lseg in segs for lm in lseg)
    nc = _get_program(segs_key, segs, w)

    in_maps = [
        {"xin": np.ascontiguousarray(xin[:, cid * BS:(cid + 1) * BS]),
         "amat": a_pack}
        for cid in range(N_CORES)
    ]
    res = run_bass_kernel_spmd(nc, in_maps, core_ids=list(range(N_CORES)))
    out_sorted = np.concatenate([res.results[cid]["out"]
                                 for cid in range(N_CORES)], axis=1)
    out = np.empty_like(out_sorted)
    out[perms[L - 1]] = out_sorted
    return out.astype(np.float32)
